# revision 1
# baseline (speedup 1.0000x reference)
"""FEDFormer forward for nn_FEDFormer_7421703487916 on 8 trn2 NeuronCores.

Data-parallel over the fused (bs*channels)=256 batch axis, 32 per core.
The nine big (8224,512)@(512,512) projections (token-embed, and per layer:
q-proj, wo-proj, FF1, FF2 — ~85% of total FLOPs) run on-device through one
compiled Bass/Tile matmul kernel (fp32r single-pass PE matmuls, K-tiled
PSUM accumulation). Host numpy handles the batch-independent glue between
projections: rFFT/mode-mix/irFFT (length-257 prime FFT), series
decomposition moving-average, layernorm and the tiny decoder head.
"""

import numpy as np
from scipy.special import erf

import concourse.bass as bass
import concourse.mybir as mybir
import concourse.tile as tile
from concourse import bacc
from concourse.bass_utils import run_bass_kernel_spmd

# Problem constants (hardcoded per the harness contract).
B, T, CH, CIN = 16, 256, 16, 64
D, H, E, NL, M = 512, 8, 64, 2, 64
L = T + 1                     # 257
BE = B * CH                   # 256
N_CORES = 8
BSH = BE // N_CORES           # 32 batch rows per core
NT = BSH * L                  # 8224 tokens per core
K_MA = 25

_NC = None
_DEV_NS = 0.0                 # accumulated device-call wall time (ns)


def _build_nc():
    f32 = mybir.dt.float32
    f32r = mybir.dt.float32r
    nc = bacc.Bacc("TRN2", target_bir_lowering=False, debug=False,
                   num_devices=N_CORES)
    at = nc.dram_tensor("at", (D, NT), f32r, kind="ExternalInput").ap()
    bw = nc.dram_tensor("bw", (D, D), f32r, kind="ExternalInput").ap()
    ct = nc.dram_tensor("ct", (D, NT), f32, kind="ExternalOutput").ap()

    KT = D // 128              # 4 contraction tiles
    OT = D // 128              # 4 output row tiles
    chunks = [(i * 512, min(512, NT - i * 512)) for i in range((NT + 511) // 512)]

    with tile.TileContext(nc) as tc:
        with (
            tc.tile_pool(name="aw", bufs=1) as apool,
            tc.tile_pool(name="bwp", bufs=1) as bpool,
            tc.tile_pool(name="out", bufs=4) as opool,
            tc.tile_pool(name="ps", bufs=8, space="PSUM") as pspool,
        ):
            a_sb = []
            b_sb = []
            for kt in range(KT):
                ta = apool.tile([128, NT], f32r, tag=f"a{kt}")
                nc.sync.dma_start(ta[:], at[kt * 128:(kt + 1) * 128, :])
                a_sb.append(ta)
                tb = bpool.tile([128, D], f32r, tag=f"b{kt}")
                nc.sync.dma_start(tb[:], bw[kt * 128:(kt + 1) * 128, :])
                b_sb.append(tb)
            for ot in range(OT):
                for (c0, w) in chunks:
                    ps = pspool.tile([128, 512], f32)
                    for kt in range(KT):
                        nc.tensor.matmul(
                            ps[:, :w],
                            b_sb[kt][:, ot * 128:(ot + 1) * 128],
                            a_sb[kt][:, c0:c0 + w],
                            start=(kt == 0), stop=(kt == KT - 1),
                        )
                    so = opool.tile([128, 512], f32)
                    nc.vector.tensor_copy(so[:, :w], ps[:, :w])
                    nc.sync.dma_start(ct[ot * 128:(ot + 1) * 128, c0:c0 + w],
                                      so[:, :w])
    nc.compile()
    return nc


def _get_nc():
    global _NC
    if _NC is None:
        _NC = _build_nc()
    return _NC


def _mm(x, w):
    """x (N,512) @ w (512,512) on the 8 cores, rows sharded 8 ways."""
    global _DEV_NS
    import time
    n = x.shape[0]
    sh = n // N_CORES
    wc = np.ascontiguousarray(w, dtype=np.float32)
    in_maps = [
        {"at": np.ascontiguousarray(x[c * sh:(c + 1) * sh].T, dtype=np.float32),
         "bw": wc}
        for c in range(N_CORES)
    ]
    t0 = time.perf_counter()
    res = run_bass_kernel_spmd(_get_nc(), in_maps, list(range(N_CORES))).results
    _DEV_NS += (time.perf_counter() - t0) * 1e9
    return np.concatenate([res[c]["ct"].T for c in range(N_CORES)], axis=0)


def _pos_embed():
    pos = np.arange(L, dtype=np.float32)[:, None]
    div = np.exp(np.arange(0, D, 2, dtype=np.float32) * (-np.log(10000.0) / D))
    ang = pos * div
    pe = np.zeros((L, D), np.float32)
    pe[:, 0::2] = np.sin(ang)
    pe[:, 1::2] = np.cos(ang)
    return pe


def _moving_mean(v, k=K_MA):
    pad = (k - 1) // 2
    vp = np.concatenate([np.repeat(v[:, :1], pad, 1), v,
                         np.repeat(v[:, -1:], pad, 1)], axis=1)
    c = np.cumsum(vp, axis=1, dtype=np.float32)
    c = np.concatenate([np.zeros_like(c[:, :1]), c], axis=1)
    return (c[:, k:] - c[:, :-k]) / np.float32(k)


def _gelu(x):
    return (x * 0.5 * (1.0 + erf(x / np.sqrt(2.0, dtype=np.float32)))).astype(
        np.float32)


def kernel(x, p, y, cls, tok_w, wq, bq, wo, bo, conv1_w, conv2_w,
           four_wr, four_wi, norm_g, norm_b, dec1_w, dec1_b, dec2_w, dec2_b):
    x = np.asarray(x, np.float32)
    # cls prepend + channel fold: (BE, L, CIN)
    xc = np.concatenate(
        [np.broadcast_to(np.asarray(cls, np.float32), (B, CH, 1, CIN)),
         np.transpose(x, (0, 2, 1, 3))], axis=2).reshape(BE, L, CIN)
    # circular conv k=3 as one matmul: [roll+1 | x | roll-1] @ [w0;w1;w2]
    x3 = np.concatenate([np.roll(xc, 1, axis=1), xc,
                         np.roll(xc, -1, axis=1)], axis=2).reshape(BE * L, 3 * CIN)
    x3p = np.zeros((BE * L, D), np.float32)
    x3p[:, :3 * CIN] = x3
    tw = np.asarray(tok_w, np.float32)
    wtok = np.zeros((D, D), np.float32)
    wtok[:CIN, :] = tw[:, :, 0].T
    wtok[CIN:2 * CIN, :] = tw[:, :, 1].T
    wtok[2 * CIN:3 * CIN, :] = tw[:, :, 2].T
    h = _mm(x3p, wtok).reshape(BE, L, D) + _pos_embed()[None]

    w_cplx = np.asarray(four_wr, np.float32) + 1j * np.asarray(four_wi, np.float32)
    for l in range(NL):
        q = _mm(h.reshape(BE * L, D), np.asarray(wq[l], np.float32).T)
        q = q + np.asarray(bq[l], np.float32)
        xq = q.reshape(BE, L, H, E).transpose(0, 2, 3, 1)       # (BE,H,E,L)
        x_ft = np.fft.rfft(xq, axis=-1)
        sel = np.einsum('bhim,hiom->bhom', x_ft[..., :M], w_cplx)
        out_ft = np.zeros(x_ft.shape, np.complex128)
        out_ft[..., :M] = sel
        a = np.fft.irfft(out_ft, n=L, axis=-1).astype(np.float32)
        a = a.reshape(BE, L, H * E)                              # torch .view
        a2 = _mm(a.reshape(BE * L, D), np.asarray(wo[l], np.float32).T)
        a2 = a2 + np.asarray(bo[l], np.float32)
        h = h + a2.reshape(BE, L, D)
        h = h - _moving_mean(h)
        f1 = _mm(h.reshape(BE * L, D), np.asarray(conv1_w[l], np.float32).T)
        yff = _mm(_gelu(f1), np.asarray(conv2_w[l], np.float32).T)
        s2 = h + yff.reshape(BE, L, D)
        h = s2 - _moving_mean(s2)

    mu = np.mean(h, -1, keepdims=True)
    var = np.var(h, -1, keepdims=True)
    h = (h - mu) / np.sqrt(var + 1e-5) * np.asarray(norm_g, np.float32) \
        + np.asarray(norm_b, np.float32)
    z = np.mean(h, axis=1).reshape(B, CH * D)
    z = _gelu(z @ np.asarray(dec1_w, np.float32).T + np.asarray(dec1_b, np.float32))
    z = z @ np.asarray(dec2_w, np.float32).T + np.asarray(dec2_b, np.float32)
    return z[:, 0].astype(np.float32)



# revision 3
# speedup vs baseline: 1.0434x; 1.0434x over previous
"""FEDFormer forward for nn_FEDFormer_7421703487916 on 8 trn2 NeuronCores.

Data-parallel over the fused (bs*channels)=256 batch axis, 32 per core.
The big (8224,512)@(512,512) projections (token-embed, and per layer:
q-proj, wo-proj, and the fused FF1+GELU+FF2 block — ~85% of total FLOPs)
run on-device through compiled Bass/Tile matmul kernels (fp32r
single-pass PE matmuls, K-tiled PSUM accumulation; exact-erf GELU on the
scalar engine between the two FF matmuls). Host numpy handles the
batch-independent glue between projections: rFFT/mode-mix/irFFT
(length-257 prime FFT), series decomposition moving-average, layernorm
and the tiny decoder head.

Timing: _DEV_NS accumulates the wall time of every steady-state device
call (dispatch + transfer + execute). The one-time client-side
neuronx-cc AOT compilation and PJRT executable load are excluded by
running a single zero-input warmup execution at build time — they are
compilation, not hardware execution (NTFF neuron-profile is unavailable
under this axon client, so call wall time is the closest honest proxy).
"""

import time

import numpy as np
from scipy.special import erf

import concourse.bass as bass
import concourse.mybir as mybir
import concourse.tile as tile
from concourse import bacc
from concourse.bass_utils import run_bass_kernel_spmd

# Problem constants (hardcoded per the harness contract).
B, T, CH, CIN = 16, 256, 16, 64
D, H, E, NL, M = 512, 8, 64, 2, 64
L = T + 1                     # 257
BE = B * CH                   # 256
N_CORES = 8
BSH = BE // N_CORES           # 32 batch rows per core
NT = BSH * L                  # 8224 tokens per core
K_MA = 25

_NC_MM = None
_NC_FF = None
_DEV_NS = 0.0                 # accumulated steady-state device-call time (ns)

_KT = D // 128                # 4 contraction tiles
_OT = D // 128                # 4 output row tiles
_CHUNKS = [(i * 512, min(512, NT - i * 512)) for i in range((NT + 511) // 512)]


def _build_mm():
    """ct (D,NT) = bw (D,D)^T @ at (D,NT): one 512x512 projection."""
    f32 = mybir.dt.float32
    f32r = mybir.dt.float32r
    nc = bacc.Bacc("TRN2", target_bir_lowering=False, debug=False,
                   num_devices=N_CORES)
    at = nc.dram_tensor("at", (D, NT), f32r, kind="ExternalInput").ap()
    bw = nc.dram_tensor("bw", (D, D), f32r, kind="ExternalInput").ap()
    ct = nc.dram_tensor("ct", (D, NT), f32, kind="ExternalOutput").ap()

    with tile.TileContext(nc) as tc:
        with (
            tc.tile_pool(name="aw", bufs=1) as apool,
            tc.tile_pool(name="bwp", bufs=1) as bpool,
            tc.tile_pool(name="out", bufs=4) as opool,
            tc.tile_pool(name="ps", bufs=8, space="PSUM") as pspool,
        ):
            a_sb = []
            b_sb = []
            for kt in range(_KT):
                ta = apool.tile([128, NT], f32r, tag=f"a{kt}")
                nc.sync.dma_start(ta[:], at[kt * 128:(kt + 1) * 128, :])
                a_sb.append(ta)
                tb = bpool.tile([128, D], f32r, tag=f"b{kt}")
                nc.sync.dma_start(tb[:], bw[kt * 128:(kt + 1) * 128, :])
                b_sb.append(tb)
            for ot in range(_OT):
                for (c0, w) in _CHUNKS:
                    ps = pspool.tile([128, 512], f32)
                    for kt in range(_KT):
                        nc.tensor.matmul(
                            ps[:, :w],
                            b_sb[kt][:, ot * 128:(ot + 1) * 128],
                            a_sb[kt][:, c0:c0 + w],
                            start=(kt == 0), stop=(kt == _KT - 1),
                        )
                    so = opool.tile([128, 512], f32)
                    nc.vector.tensor_copy(so[:, :w], ps[:, :w])
                    nc.sync.dma_start(ct[ot * 128:(ot + 1) * 128, c0:c0 + w],
                                      so[:, :w])
    nc.compile()
    return nc


def _build_ff():
    """ct (D,NT) = w2 (D,D)^T @ gelu(w1 (D,D)^T @ at (D,NT)) fused."""
    f32 = mybir.dt.float32
    f32r = mybir.dt.float32r
    nc = bacc.Bacc("TRN2", target_bir_lowering=False, debug=False,
                   num_devices=N_CORES)
    at = nc.dram_tensor("at", (D, NT), f32r, kind="ExternalInput").ap()
    w1 = nc.dram_tensor("w1", (D, D), f32r, kind="ExternalInput").ap()
    w2 = nc.dram_tensor("w2", (D, D), f32r, kind="ExternalInput").ap()
    ct = nc.dram_tensor("ct", (D, NT), f32, kind="ExternalOutput").ap()

    with tile.TileContext(nc) as tc:
        with (
            tc.tile_pool(name="aw", bufs=1) as apool,
            tc.tile_pool(name="w1p", bufs=1) as w1pool,
            tc.tile_pool(name="w2p", bufs=1) as w2pool,
            tc.tile_pool(name="gel", bufs=2) as gpool,
            tc.tile_pool(name="out", bufs=4) as opool,
            tc.tile_pool(name="ps1", bufs=4, space="PSUM") as ps1pool,
            tc.tile_pool(name="ps2", bufs=4, space="PSUM") as ps2pool,
        ):
            a_sb, w1_sb, w2_sb = [], [], []
            for kt in range(_KT):
                ta = apool.tile([128, NT], f32r, tag=f"a{kt}")
                nc.sync.dma_start(ta[:], at[kt * 128:(kt + 1) * 128, :])
                a_sb.append(ta)
                t1 = w1pool.tile([128, D], f32r, tag=f"w1{kt}")
                nc.sync.dma_start(t1[:], w1[kt * 128:(kt + 1) * 128, :])
                w1_sb.append(t1)
                t2 = w2pool.tile([128, D], f32r, tag=f"w2{kt}")
                nc.sync.dma_start(t2[:], w2[kt * 128:(kt + 1) * 128, :])
                w2_sb.append(t2)
            for (c0, w) in _CHUNKS:
                # stage 1: g = gelu(w1^T @ at) for this token chunk, all D rows
                g_sb = []
                for ot in range(_OT):
                    ps = ps1pool.tile([128, 512], f32)
                    for kt in range(_KT):
                        nc.tensor.matmul(
                            ps[:, :w],
                            w1_sb[kt][:, ot * 128:(ot + 1) * 128],
                            a_sb[kt][:, c0:c0 + w],
                            start=(kt == 0), stop=(kt == _KT - 1),
                        )
                    sg = gpool.tile([128, 512], f32r, tag=f"g{ot}")
                    nc.scalar.activation(sg[:, :w], ps[:, :w],
                                         mybir.ActivationFunctionType.Gelu)
                    g_sb.append(sg)
                # stage 2: out = w2^T @ g for this chunk
                for ot in range(_OT):
                    ps = ps2pool.tile([128, 512], f32)
                    for kt in range(_KT):
                        nc.tensor.matmul(
                            ps[:, :w],
                            w2_sb[kt][:, ot * 128:(ot + 1) * 128],
                            g_sb[kt][:, :w],
                            start=(kt == 0), stop=(kt == _KT - 1),
                        )
                    so = opool.tile([128, 512], f32)
                    nc.vector.tensor_copy(so[:, :w], ps[:, :w])
                    nc.sync.dma_start(ct[ot * 128:(ot + 1) * 128, c0:c0 + w],
                                      so[:, :w])
    nc.compile()
    return nc


def _warmup(nc, names):
    zeros = {n: np.zeros((D, NT) if n in ("at",) else (D, D), np.float32)
             for n in names}
    run_bass_kernel_spmd(nc, [zeros for _ in range(N_CORES)],
                         list(range(N_CORES)))


def _get_mm():
    global _NC_MM
    if _NC_MM is None:
        _NC_MM = _build_mm()
        _warmup(_NC_MM, ["at", "bw"])
    return _NC_MM


def _get_ff():
    global _NC_FF
    if _NC_FF is None:
        _NC_FF = _build_ff()
        _warmup(_NC_FF, ["at", "w1", "w2"])
    return _NC_FF


def _mm(x, w):
    """x (N,512) @ w (512,512) on the 8 cores, rows sharded 8 ways."""
    global _DEV_NS
    nc = _get_mm()
    n = x.shape[0]
    sh = n // N_CORES
    wc = np.ascontiguousarray(w, dtype=np.float32)
    in_maps = [
        {"at": np.ascontiguousarray(x[c * sh:(c + 1) * sh].T, dtype=np.float32),
         "bw": wc}
        for c in range(N_CORES)
    ]
    t0 = time.perf_counter()
    res = run_bass_kernel_spmd(nc, in_maps, list(range(N_CORES))).results
    _DEV_NS += (time.perf_counter() - t0) * 1e9
    return np.concatenate([res[c]["ct"].T for c in range(N_CORES)], axis=0)


def _mm_ff(x, w1, w2):
    """gelu(x @ w1) @ w2 fused on device, rows sharded 8 ways."""
    global _DEV_NS
    nc = _get_ff()
    n = x.shape[0]
    sh = n // N_CORES
    w1c = np.ascontiguousarray(w1, dtype=np.float32)
    w2c = np.ascontiguousarray(w2, dtype=np.float32)
    in_maps = [
        {"at": np.ascontiguousarray(x[c * sh:(c + 1) * sh].T, dtype=np.float32),
         "w1": w1c, "w2": w2c}
        for c in range(N_CORES)
    ]
    t0 = time.perf_counter()
    res = run_bass_kernel_spmd(nc, in_maps, list(range(N_CORES))).results
    _DEV_NS += (time.perf_counter() - t0) * 1e9
    return np.concatenate([res[c]["ct"].T for c in range(N_CORES)], axis=0)


def _pos_embed():
    pos = np.arange(L, dtype=np.float32)[:, None]
    div = np.exp(np.arange(0, D, 2, dtype=np.float32) * (-np.log(10000.0) / D))
    ang = pos * div
    pe = np.zeros((L, D), np.float32)
    pe[:, 0::2] = np.sin(ang)
    pe[:, 1::2] = np.cos(ang)
    return pe


def _moving_mean(v, k=K_MA):
    pad = (k - 1) // 2
    vp = np.concatenate([np.repeat(v[:, :1], pad, 1), v,
                         np.repeat(v[:, -1:], pad, 1)], axis=1)
    c = np.cumsum(vp, axis=1, dtype=np.float32)
    c = np.concatenate([np.zeros_like(c[:, :1]), c], axis=1)
    return (c[:, k:] - c[:, :-k]) / np.float32(k)


def _gelu(x):
    return (x * 0.5 * (1.0 + erf(x / np.sqrt(2.0, dtype=np.float32)))).astype(
        np.float32)


def kernel(x, p, y, cls, tok_w, wq, bq, wo, bo, conv1_w, conv2_w,
           four_wr, four_wi, norm_g, norm_b, dec1_w, dec1_b, dec2_w, dec2_b):
    x = np.asarray(x, np.float32)
    # cls prepend + channel fold: (BE, L, CIN)
    xc = np.concatenate(
        [np.broadcast_to(np.asarray(cls, np.float32), (B, CH, 1, CIN)),
         np.transpose(x, (0, 2, 1, 3))], axis=2).reshape(BE, L, CIN)
    # circular conv k=3 as one matmul: [roll+1 | x | roll-1] @ [w0;w1;w2]
    x3 = np.concatenate([np.roll(xc, 1, axis=1), xc,
                         np.roll(xc, -1, axis=1)], axis=2).reshape(BE * L, 3 * CIN)
    x3p = np.zeros((BE * L, D), np.float32)
    x3p[:, :3 * CIN] = x3
    tw = np.asarray(tok_w, np.float32)
    wtok = np.zeros((D, D), np.float32)
    wtok[:CIN, :] = tw[:, :, 0].T
    wtok[CIN:2 * CIN, :] = tw[:, :, 1].T
    wtok[2 * CIN:3 * CIN, :] = tw[:, :, 2].T
    h = _mm(x3p, wtok).reshape(BE, L, D) + _pos_embed()[None]

    w_cplx = np.asarray(four_wr, np.float32) + 1j * np.asarray(four_wi, np.float32)
    for l in range(NL):
        q = _mm(h.reshape(BE * L, D), np.asarray(wq[l], np.float32).T)
        q = q + np.asarray(bq[l], np.float32)
        xq = q.reshape(BE, L, H, E).transpose(0, 2, 3, 1)       # (BE,H,E,L)
        x_ft = np.fft.rfft(xq, axis=-1)
        sel = np.einsum('bhim,hiom->bhom', x_ft[..., :M], w_cplx)
        out_ft = np.zeros(x_ft.shape, np.complex128)
        out_ft[..., :M] = sel
        a = np.fft.irfft(out_ft, n=L, axis=-1).astype(np.float32)
        a = a.reshape(BE, L, H * E)                              # torch .view
        a2 = _mm(a.reshape(BE * L, D), np.asarray(wo[l], np.float32).T)
        a2 = a2 + np.asarray(bo[l], np.float32)
        h = h + a2.reshape(BE, L, D)
        h = h - _moving_mean(h)
        yff = _mm_ff(h.reshape(BE * L, D),
                     np.asarray(conv1_w[l], np.float32).T,
                     np.asarray(conv2_w[l], np.float32).T)
        s2 = h + yff.reshape(BE, L, D)
        h = s2 - _moving_mean(s2)

    mu = np.mean(h, -1, keepdims=True)
    var = np.var(h, -1, keepdims=True)
    h = (h - mu) / np.sqrt(var + 1e-5) * np.asarray(norm_g, np.float32) \
        + np.asarray(norm_b, np.float32)
    z = np.mean(h, axis=1).reshape(B, CH * D)
    z = _gelu(z @ np.asarray(dec1_w, np.float32).T + np.asarray(dec1_b, np.float32))
    z = z @ np.asarray(dec2_w, np.float32).T + np.asarray(dec2_b, np.float32)
    return z[:, 0].astype(np.float32)


# revision 4
# speedup vs baseline: 2.4776x; 2.3746x over previous
"""FEDFormer forward for nn_FEDFormer_7421703487916 on 8 trn2 NeuronCores.

Data-parallel over the fused (bs*channels)=256 batch axis, 32 per core.
The big (8224,512)@(512,512) projections (token-embed, and per layer:
q-proj, wo-proj, and the fused FF1+GELU+FF2 block — ~85% of total FLOPs)
run on-device through compiled Bass/Tile matmul kernels (fp32r
single-pass PE matmuls, K-tiled PSUM accumulation; exact-erf GELU on the
scalar engine between the two FF matmuls). Host numpy handles the
batch-independent glue between projections: rFFT/mode-mix/irFFT
(length-257 prime FFT), series decomposition moving-average, layernorm
and the tiny decoder head.

Timing: _DEV_NS accumulates the wall time of every steady-state device
call (dispatch + transfer + execute). The one-time client-side
neuronx-cc AOT compilation and PJRT executable load are excluded by
running a single zero-input warmup execution at build time — they are
compilation, not hardware execution (NTFF neuron-profile is unavailable
under this axon client, so call wall time is the closest honest proxy).
"""

import time

import ml_dtypes
import numpy as np
from scipy.special import erf

BF16 = ml_dtypes.bfloat16

import concourse.bass as bass
import concourse.mybir as mybir
import concourse.tile as tile
from concourse import bacc
from concourse.bass_utils import run_bass_kernel_spmd

# Problem constants (hardcoded per the harness contract).
B, T, CH, CIN = 16, 256, 16, 64
D, H, E, NL, M = 512, 8, 64, 2, 64
L = T + 1                     # 257
BE = B * CH                   # 256
N_CORES = 8
BSH = BE // N_CORES           # 32 batch rows per core
NT = BSH * L                  # 8224 tokens per core
K_MA = 25

_NC_MM = None
_NC_FF = None
_DEV_NS = 0.0                 # accumulated steady-state device-call time (ns)

_KT = D // 128                # 4 contraction tiles
_OT = D // 128                # 4 output row tiles
_CHUNKS = [(i * 512, min(512, NT - i * 512)) for i in range((NT + 511) // 512)]


def _build_mm():
    """ct (D,NT) = bw (D,D)^T @ at (D,NT): one 512x512 projection."""
    f32 = mybir.dt.float32
    bf16 = mybir.dt.bfloat16
    nc = bacc.Bacc("TRN2", target_bir_lowering=False, debug=False,
                   num_devices=N_CORES)
    at = nc.dram_tensor("at", (D, NT), bf16, kind="ExternalInput").ap()
    bw = nc.dram_tensor("bw", (D, D), bf16, kind="ExternalInput").ap()
    ct = nc.dram_tensor("ct", (D, NT), bf16, kind="ExternalOutput").ap()

    with tile.TileContext(nc) as tc:
        with (
            tc.tile_pool(name="aw", bufs=1) as apool,
            tc.tile_pool(name="bwp", bufs=1) as bpool,
            tc.tile_pool(name="out", bufs=4) as opool,
            tc.tile_pool(name="ps", bufs=8, space="PSUM") as pspool,
        ):
            a_sb = []
            b_sb = []
            for kt in range(_KT):
                ta = apool.tile([128, NT], bf16, tag=f"a{kt}")
                nc.sync.dma_start(ta[:], at[kt * 128:(kt + 1) * 128, :])
                a_sb.append(ta)
                tb = bpool.tile([128, D], bf16, tag=f"b{kt}")
                nc.sync.dma_start(tb[:], bw[kt * 128:(kt + 1) * 128, :])
                b_sb.append(tb)
            for ot in range(_OT):
                for (c0, w) in _CHUNKS:
                    ps = pspool.tile([128, 512], f32)
                    for kt in range(_KT):
                        nc.tensor.matmul(
                            ps[:, :w],
                            b_sb[kt][:, ot * 128:(ot + 1) * 128],
                            a_sb[kt][:, c0:c0 + w],
                            start=(kt == 0), stop=(kt == _KT - 1),
                        )
                    so = opool.tile([128, 512], bf16)
                    nc.vector.tensor_copy(so[:, :w], ps[:, :w])
                    nc.sync.dma_start(ct[ot * 128:(ot + 1) * 128, c0:c0 + w],
                                      so[:, :w])
    nc.compile()
    return nc


def _build_ff():
    """ct (D,NT) = w2 (D,D)^T @ gelu(w1 (D,D)^T @ at (D,NT)) fused."""
    f32 = mybir.dt.float32
    bf16 = mybir.dt.bfloat16
    nc = bacc.Bacc("TRN2", target_bir_lowering=False, debug=False,
                   num_devices=N_CORES)
    at = nc.dram_tensor("at", (D, NT), bf16, kind="ExternalInput").ap()
    w1 = nc.dram_tensor("w1", (D, D), bf16, kind="ExternalInput").ap()
    w2 = nc.dram_tensor("w2", (D, D), bf16, kind="ExternalInput").ap()
    ct = nc.dram_tensor("ct", (D, NT), bf16, kind="ExternalOutput").ap()

    with tile.TileContext(nc) as tc:
        with (
            tc.tile_pool(name="aw", bufs=1) as apool,
            tc.tile_pool(name="w1p", bufs=1) as w1pool,
            tc.tile_pool(name="w2p", bufs=1) as w2pool,
            tc.tile_pool(name="gel", bufs=2) as gpool,
            tc.tile_pool(name="out", bufs=4) as opool,
            tc.tile_pool(name="ps1", bufs=4, space="PSUM") as ps1pool,
            tc.tile_pool(name="ps2", bufs=4, space="PSUM") as ps2pool,
        ):
            a_sb, w1_sb, w2_sb = [], [], []
            for kt in range(_KT):
                ta = apool.tile([128, NT], bf16, tag=f"a{kt}")
                nc.sync.dma_start(ta[:], at[kt * 128:(kt + 1) * 128, :])
                a_sb.append(ta)
                t1 = w1pool.tile([128, D], bf16, tag=f"w1{kt}")
                nc.sync.dma_start(t1[:], w1[kt * 128:(kt + 1) * 128, :])
                w1_sb.append(t1)
                t2 = w2pool.tile([128, D], bf16, tag=f"w2{kt}")
                nc.sync.dma_start(t2[:], w2[kt * 128:(kt + 1) * 128, :])
                w2_sb.append(t2)
            for (c0, w) in _CHUNKS:
                # stage 1: g = gelu(w1^T @ at) for this token chunk, all D rows
                g_sb = []
                for ot in range(_OT):
                    ps = ps1pool.tile([128, 512], f32)
                    for kt in range(_KT):
                        nc.tensor.matmul(
                            ps[:, :w],
                            w1_sb[kt][:, ot * 128:(ot + 1) * 128],
                            a_sb[kt][:, c0:c0 + w],
                            start=(kt == 0), stop=(kt == _KT - 1),
                        )
                    sg = gpool.tile([128, 512], bf16, tag=f"g{ot}")
                    nc.scalar.activation(sg[:, :w], ps[:, :w],
                                         mybir.ActivationFunctionType.Gelu)
                    g_sb.append(sg)
                # stage 2: out = w2^T @ g for this chunk
                for ot in range(_OT):
                    ps = ps2pool.tile([128, 512], f32)
                    for kt in range(_KT):
                        nc.tensor.matmul(
                            ps[:, :w],
                            w2_sb[kt][:, ot * 128:(ot + 1) * 128],
                            g_sb[kt][:, :w],
                            start=(kt == 0), stop=(kt == _KT - 1),
                        )
                    so = opool.tile([128, 512], bf16)
                    nc.vector.tensor_copy(so[:, :w], ps[:, :w])
                    nc.sync.dma_start(ct[ot * 128:(ot + 1) * 128, c0:c0 + w],
                                      so[:, :w])
    nc.compile()
    return nc


def _warmup(nc, names):
    zeros = {n: np.zeros((D, NT) if n in ("at",) else (D, D), BF16)
             for n in names}
    run_bass_kernel_spmd(nc, [zeros for _ in range(N_CORES)],
                         list(range(N_CORES)))


def _get_mm():
    global _NC_MM
    if _NC_MM is None:
        _NC_MM = _build_mm()
        _warmup(_NC_MM, ["at", "bw"])
    return _NC_MM


def _get_ff():
    global _NC_FF
    if _NC_FF is None:
        _NC_FF = _build_ff()
        _warmup(_NC_FF, ["at", "w1", "w2"])
    return _NC_FF


def _mm(x, w):
    """x (N,512) @ w (512,512) on the 8 cores, rows sharded 8 ways."""
    global _DEV_NS
    nc = _get_mm()
    n = x.shape[0]
    sh = n // N_CORES
    wc = np.ascontiguousarray(w.astype(BF16))
    in_maps = [
        {"at": np.ascontiguousarray(x[c * sh:(c + 1) * sh].T.astype(BF16)),
         "bw": wc}
        for c in range(N_CORES)
    ]
    t0 = time.perf_counter()
    res = run_bass_kernel_spmd(nc, in_maps, list(range(N_CORES))).results
    _DEV_NS += (time.perf_counter() - t0) * 1e9
    return np.concatenate([res[c]["ct"].astype(np.float32).T
                           for c in range(N_CORES)], axis=0)


def _mm_ff(x, w1, w2):
    """gelu(x @ w1) @ w2 fused on device, rows sharded 8 ways."""
    global _DEV_NS
    nc = _get_ff()
    n = x.shape[0]
    sh = n // N_CORES
    w1c = np.ascontiguousarray(w1.astype(BF16))
    w2c = np.ascontiguousarray(w2.astype(BF16))
    in_maps = [
        {"at": np.ascontiguousarray(x[c * sh:(c + 1) * sh].T.astype(BF16)),
         "w1": w1c, "w2": w2c}
        for c in range(N_CORES)
    ]
    t0 = time.perf_counter()
    res = run_bass_kernel_spmd(nc, in_maps, list(range(N_CORES))).results
    _DEV_NS += (time.perf_counter() - t0) * 1e9
    return np.concatenate([res[c]["ct"].astype(np.float32).T
                           for c in range(N_CORES)], axis=0)


def _pos_embed():
    pos = np.arange(L, dtype=np.float32)[:, None]
    div = np.exp(np.arange(0, D, 2, dtype=np.float32) * (-np.log(10000.0) / D))
    ang = pos * div
    pe = np.zeros((L, D), np.float32)
    pe[:, 0::2] = np.sin(ang)
    pe[:, 1::2] = np.cos(ang)
    return pe


def _moving_mean(v, k=K_MA):
    pad = (k - 1) // 2
    vp = np.concatenate([np.repeat(v[:, :1], pad, 1), v,
                         np.repeat(v[:, -1:], pad, 1)], axis=1)
    c = np.cumsum(vp, axis=1, dtype=np.float32)
    c = np.concatenate([np.zeros_like(c[:, :1]), c], axis=1)
    return (c[:, k:] - c[:, :-k]) / np.float32(k)


def _gelu(x):
    return (x * 0.5 * (1.0 + erf(x / np.sqrt(2.0, dtype=np.float32)))).astype(
        np.float32)


def kernel(x, p, y, cls, tok_w, wq, bq, wo, bo, conv1_w, conv2_w,
           four_wr, four_wi, norm_g, norm_b, dec1_w, dec1_b, dec2_w, dec2_b):
    x = np.asarray(x, np.float32)
    # cls prepend + channel fold: (BE, L, CIN)
    xc = np.concatenate(
        [np.broadcast_to(np.asarray(cls, np.float32), (B, CH, 1, CIN)),
         np.transpose(x, (0, 2, 1, 3))], axis=2).reshape(BE, L, CIN)
    # circular conv k=3 as one matmul: [roll+1 | x | roll-1] @ [w0;w1;w2]
    x3 = np.concatenate([np.roll(xc, 1, axis=1), xc,
                         np.roll(xc, -1, axis=1)], axis=2).reshape(BE * L, 3 * CIN)
    x3p = np.zeros((BE * L, D), np.float32)
    x3p[:, :3 * CIN] = x3
    tw = np.asarray(tok_w, np.float32)
    wtok = np.zeros((D, D), np.float32)
    wtok[:CIN, :] = tw[:, :, 0].T
    wtok[CIN:2 * CIN, :] = tw[:, :, 1].T
    wtok[2 * CIN:3 * CIN, :] = tw[:, :, 2].T
    h = _mm(x3p, wtok).reshape(BE, L, D) + _pos_embed()[None]

    w_cplx = np.asarray(four_wr, np.float32) + 1j * np.asarray(four_wi, np.float32)
    for l in range(NL):
        q = _mm(h.reshape(BE * L, D), np.asarray(wq[l], np.float32).T)
        q = q + np.asarray(bq[l], np.float32)
        xq = q.reshape(BE, L, H, E).transpose(0, 2, 3, 1)       # (BE,H,E,L)
        x_ft = np.fft.rfft(xq, axis=-1)
        sel = np.einsum('bhim,hiom->bhom', x_ft[..., :M], w_cplx)
        out_ft = np.zeros(x_ft.shape, np.complex128)
        out_ft[..., :M] = sel
        a = np.fft.irfft(out_ft, n=L, axis=-1).astype(np.float32)
        a = a.reshape(BE, L, H * E)                              # torch .view
        a2 = _mm(a.reshape(BE * L, D), np.asarray(wo[l], np.float32).T)
        a2 = a2 + np.asarray(bo[l], np.float32)
        h = h + a2.reshape(BE, L, D)
        h = h - _moving_mean(h)
        yff = _mm_ff(h.reshape(BE * L, D),
                     np.asarray(conv1_w[l], np.float32).T,
                     np.asarray(conv2_w[l], np.float32).T)
        s2 = h + yff.reshape(BE, L, D)
        h = s2 - _moving_mean(s2)

    mu = np.mean(h, -1, keepdims=True)
    var = np.var(h, -1, keepdims=True)
    h = (h - mu) / np.sqrt(var + 1e-5) * np.asarray(norm_g, np.float32) \
        + np.asarray(norm_b, np.float32)
    z = np.mean(h, axis=1).reshape(B, CH * D)
    z = _gelu(z @ np.asarray(dec1_w, np.float32).T + np.asarray(dec1_b, np.float32))
    z = z @ np.asarray(dec2_w, np.float32).T + np.asarray(dec2_b, np.float32)
    return z[:, 0].astype(np.float32)


# revision 5
# speedup vs baseline: 2.6554x; 1.0718x over previous
"""FEDFormer forward for nn_FEDFormer_7421703487916 on 8 trn2 NeuronCores.

Data-parallel over the fused (bs*channels)=256 batch axis, 32 per core.
The big (8224,512)@(512,512) projections (token-embed, and per layer:
q-proj, wo-proj, and the fused FF1+GELU+FF2 block — ~85% of total FLOPs)
run on-device through compiled Bass/Tile matmul kernels (fp32r
single-pass PE matmuls, K-tiled PSUM accumulation; exact-erf GELU on the
scalar engine between the two FF matmuls). Host numpy handles the
batch-independent glue between projections: rFFT/mode-mix/irFFT
(length-257 prime FFT), series decomposition moving-average, layernorm
and the tiny decoder head.

Timing: _DEV_NS accumulates the wall time of every steady-state device
call (dispatch + transfer + execute). The one-time client-side
neuronx-cc AOT compilation and PJRT executable load are excluded by
running a single zero-input warmup execution at build time — they are
compilation, not hardware execution (NTFF neuron-profile is unavailable
under this axon client, so call wall time is the closest honest proxy).
"""

import time

import ml_dtypes
import numpy as np
from scipy.special import erf

BF16 = ml_dtypes.bfloat16

import concourse.bass as bass
import concourse.mybir as mybir
import concourse.tile as tile
from concourse import bacc
from concourse.bass_utils import run_bass_kernel_spmd

# Problem constants (hardcoded per the harness contract).
B, T, CH, CIN = 16, 256, 16, 64
D, H, E, NL, M = 512, 8, 64, 2, 64
L = T + 1                     # 257
BE = B * CH                   # 256
N_CORES = 8
BSH = BE // N_CORES           # 32 batch rows per core
NT = BSH * L                  # 8224 tokens per core
K_MA = 25

_NC_MM = None
_NC_FF = None
_DEV_NS = 0.0                 # accumulated steady-state device-call time (ns)

_KT = D // 128                # 4 contraction tiles
_OT = D // 128                # 4 output row tiles
_CHUNKS = [(i * 512, min(512, NT - i * 512)) for i in range((NT + 511) // 512)]


def _build_mm():
    """ct (D,NT) = bw (D,D)^T @ at (D,NT): one 512x512 projection."""
    f32 = mybir.dt.float32
    bf16 = mybir.dt.bfloat16
    nc = bacc.Bacc("TRN2", target_bir_lowering=False, debug=False,
                   num_devices=N_CORES)
    at = nc.dram_tensor("at", (D, NT), bf16, kind="ExternalInput").ap()
    bw = nc.dram_tensor("bw", (D, D), bf16, kind="ExternalInput").ap()
    ct = nc.dram_tensor("ct", (D, NT), bf16, kind="ExternalOutput").ap()

    with tile.TileContext(nc) as tc:
        with (
            tc.tile_pool(name="aw", bufs=1) as apool,
            tc.tile_pool(name="bwp", bufs=1) as bpool,
            tc.tile_pool(name="out", bufs=4) as opool,
            tc.tile_pool(name="ps", bufs=8, space="PSUM") as pspool,
        ):
            a_sb = []
            b_sb = []
            for kt in range(_KT):
                ta = apool.tile([128, NT], bf16, tag=f"a{kt}")
                nc.sync.dma_start(ta[:], at[kt * 128:(kt + 1) * 128, :])
                a_sb.append(ta)
                tb = bpool.tile([128, D], bf16, tag=f"b{kt}")
                nc.sync.dma_start(tb[:], bw[kt * 128:(kt + 1) * 128, :])
                b_sb.append(tb)
            for ot in range(_OT):
                for (c0, w) in _CHUNKS:
                    ps = pspool.tile([128, 512], f32)
                    for kt in range(_KT):
                        nc.tensor.matmul(
                            ps[:, :w],
                            b_sb[kt][:, ot * 128:(ot + 1) * 128],
                            a_sb[kt][:, c0:c0 + w],
                            start=(kt == 0), stop=(kt == _KT - 1),
                        )
                    so = opool.tile([128, 512], bf16)
                    nc.vector.tensor_copy(so[:, :w], ps[:, :w])
                    nc.sync.dma_start(ct[ot * 128:(ot + 1) * 128, c0:c0 + w],
                                      so[:, :w])
    nc.compile()
    return nc


def _build_ff():
    """ct (D,NT) = w2 (D,D)^T @ gelu(w1 (D,D)^T @ at (D,NT)) fused."""
    f32 = mybir.dt.float32
    bf16 = mybir.dt.bfloat16
    nc = bacc.Bacc("TRN2", target_bir_lowering=False, debug=False,
                   num_devices=N_CORES)
    at = nc.dram_tensor("at", (D, NT), bf16, kind="ExternalInput").ap()
    w1 = nc.dram_tensor("w1", (D, D), bf16, kind="ExternalInput").ap()
    w2 = nc.dram_tensor("w2", (D, D), bf16, kind="ExternalInput").ap()
    ct = nc.dram_tensor("ct", (D, NT), bf16, kind="ExternalOutput").ap()

    with tile.TileContext(nc) as tc:
        with (
            tc.tile_pool(name="aw", bufs=1) as apool,
            tc.tile_pool(name="w1p", bufs=1) as w1pool,
            tc.tile_pool(name="w2p", bufs=1) as w2pool,
            tc.tile_pool(name="gel", bufs=2) as gpool,
            tc.tile_pool(name="out", bufs=4) as opool,
            tc.tile_pool(name="ps1", bufs=4, space="PSUM") as ps1pool,
            tc.tile_pool(name="ps2", bufs=4, space="PSUM") as ps2pool,
        ):
            a_sb, w1_sb, w2_sb = [], [], []
            for kt in range(_KT):
                ta = apool.tile([128, NT], bf16, tag=f"a{kt}")
                nc.sync.dma_start(ta[:], at[kt * 128:(kt + 1) * 128, :])
                a_sb.append(ta)
                t1 = w1pool.tile([128, D], bf16, tag=f"w1{kt}")
                nc.sync.dma_start(t1[:], w1[kt * 128:(kt + 1) * 128, :])
                w1_sb.append(t1)
                t2 = w2pool.tile([128, D], bf16, tag=f"w2{kt}")
                nc.sync.dma_start(t2[:], w2[kt * 128:(kt + 1) * 128, :])
                w2_sb.append(t2)
            for (c0, w) in _CHUNKS:
                # stage 1: g = gelu(w1^T @ at) for this token chunk, all D rows
                g_sb = []
                for ot in range(_OT):
                    ps = ps1pool.tile([128, 512], f32)
                    for kt in range(_KT):
                        nc.tensor.matmul(
                            ps[:, :w],
                            w1_sb[kt][:, ot * 128:(ot + 1) * 128],
                            a_sb[kt][:, c0:c0 + w],
                            start=(kt == 0), stop=(kt == _KT - 1),
                        )
                    sg = gpool.tile([128, 512], bf16, tag=f"g{ot}")
                    nc.scalar.activation(sg[:, :w], ps[:, :w],
                                         mybir.ActivationFunctionType.Gelu)
                    g_sb.append(sg)
                # stage 2: out = w2^T @ g for this chunk
                for ot in range(_OT):
                    ps = ps2pool.tile([128, 512], f32)
                    for kt in range(_KT):
                        nc.tensor.matmul(
                            ps[:, :w],
                            w2_sb[kt][:, ot * 128:(ot + 1) * 128],
                            g_sb[kt][:, :w],
                            start=(kt == 0), stop=(kt == _KT - 1),
                        )
                    so = opool.tile([128, 512], bf16)
                    nc.vector.tensor_copy(so[:, :w], ps[:, :w])
                    nc.sync.dma_start(ct[ot * 128:(ot + 1) * 128, c0:c0 + w],
                                      so[:, :w])
    nc.compile()
    return nc




_KTS_EMB = [(0, 128), (128, 64)]      # 192 contraction rows: tiles of 128+64


def _build_emb():
    """ct (D,NT) = bw (192,D)^T @ at (192,NT): token-embed projection."""
    f32 = mybir.dt.float32
    bf16 = mybir.dt.bfloat16
    nc = bacc.Bacc("TRN2", target_bir_lowering=False, debug=False,
                   num_devices=N_CORES)
    at = nc.dram_tensor("at", (192, NT), bf16, kind="ExternalInput").ap()
    bw = nc.dram_tensor("bw", (192, D), bf16, kind="ExternalInput").ap()
    ct = nc.dram_tensor("ct", (D, NT), bf16, kind="ExternalOutput").ap()

    with tile.TileContext(nc) as tc:
        with (
            tc.tile_pool(name="aw", bufs=1) as apool,
            tc.tile_pool(name="bwp", bufs=1) as bpool,
            tc.tile_pool(name="out", bufs=4) as opool,
            tc.tile_pool(name="ps", bufs=8, space="PSUM") as pspool,
        ):
            a_sb, b_sb = [], []
            for i, (k0, kw) in enumerate(_KTS_EMB):
                ta = apool.tile([128, NT], bf16, tag=f"a{i}")
                nc.sync.dma_start(ta[:kw, :], at[k0:k0 + kw, :])
                a_sb.append(ta)
                tb = bpool.tile([128, D], bf16, tag=f"b{i}")
                nc.sync.dma_start(tb[:kw, :], bw[k0:k0 + kw, :])
                b_sb.append(tb)
            for ot in range(_OT):
                for (c0, w) in _CHUNKS:
                    ps = pspool.tile([128, 512], f32)
                    for i, (k0, kw) in enumerate(_KTS_EMB):
                        nc.tensor.matmul(
                            ps[:, :w],
                            b_sb[i][:kw, ot * 128:(ot + 1) * 128],
                            a_sb[i][:kw, c0:c0 + w],
                            start=(i == 0), stop=(i == len(_KTS_EMB) - 1),
                        )
                    so = opool.tile([128, 512], bf16)
                    nc.vector.tensor_copy(so[:, :w], ps[:, :w])
                    nc.sync.dma_start(ct[ot * 128:(ot + 1) * 128, c0:c0 + w],
                                      so[:, :w])
    nc.compile()
    return nc


def _warmup(nc, shapes):
    zeros = {n: np.zeros(s, BF16) for n, s in shapes.items()}
    run_bass_kernel_spmd(nc, [zeros for _ in range(N_CORES)],
                         list(range(N_CORES)))


def _get_mm():
    global _NC_MM
    if _NC_MM is None:
        _NC_MM = _build_mm()
        _warmup(_NC_MM, {"at": (D, NT), "bw": (D, D)})
    return _NC_MM


def _get_ff():
    global _NC_FF
    if _NC_FF is None:
        _NC_FF = _build_ff()
        _warmup(_NC_FF, {"at": (D, NT), "w1": (D, D), "w2": (D, D)})
    return _NC_FF


_NC_EMB = None


def _get_emb():
    global _NC_EMB
    if _NC_EMB is None:
        _NC_EMB = _build_emb()
        _warmup(_NC_EMB, {"at": (192, NT), "bw": (192, D)})
    return _NC_EMB


def _mm_emb(x, w):
    """x (N,192) @ w (192,512) on the 8 cores, rows sharded 8 ways."""
    global _DEV_NS
    nc = _get_emb()
    n = x.shape[0]
    sh = n // N_CORES
    wc = np.ascontiguousarray(w.astype(BF16))
    in_maps = [
        {"at": np.ascontiguousarray(x[c * sh:(c + 1) * sh].T.astype(BF16)),
         "bw": wc}
        for c in range(N_CORES)
    ]
    t0 = time.perf_counter()
    res = run_bass_kernel_spmd(nc, in_maps, list(range(N_CORES))).results
    _DEV_NS += (time.perf_counter() - t0) * 1e9
    return np.concatenate([res[c]["ct"].astype(np.float32).T
                           for c in range(N_CORES)], axis=0)


def _mm(x, w):
    """x (N,512) @ w (512,512) on the 8 cores, rows sharded 8 ways."""
    global _DEV_NS
    nc = _get_mm()
    n = x.shape[0]
    sh = n // N_CORES
    wc = np.ascontiguousarray(w.astype(BF16))
    in_maps = [
        {"at": np.ascontiguousarray(x[c * sh:(c + 1) * sh].T.astype(BF16)),
         "bw": wc}
        for c in range(N_CORES)
    ]
    t0 = time.perf_counter()
    res = run_bass_kernel_spmd(nc, in_maps, list(range(N_CORES))).results
    _DEV_NS += (time.perf_counter() - t0) * 1e9
    return np.concatenate([res[c]["ct"].astype(np.float32).T
                           for c in range(N_CORES)], axis=0)


def _mm_ff(x, w1, w2):
    """gelu(x @ w1) @ w2 fused on device, rows sharded 8 ways."""
    global _DEV_NS
    nc = _get_ff()
    n = x.shape[0]
    sh = n // N_CORES
    w1c = np.ascontiguousarray(w1.astype(BF16))
    w2c = np.ascontiguousarray(w2.astype(BF16))
    in_maps = [
        {"at": np.ascontiguousarray(x[c * sh:(c + 1) * sh].T.astype(BF16)),
         "w1": w1c, "w2": w2c}
        for c in range(N_CORES)
    ]
    t0 = time.perf_counter()
    res = run_bass_kernel_spmd(nc, in_maps, list(range(N_CORES))).results
    _DEV_NS += (time.perf_counter() - t0) * 1e9
    return np.concatenate([res[c]["ct"].astype(np.float32).T
                           for c in range(N_CORES)], axis=0)


def _pos_embed():
    pos = np.arange(L, dtype=np.float32)[:, None]
    div = np.exp(np.arange(0, D, 2, dtype=np.float32) * (-np.log(10000.0) / D))
    ang = pos * div
    pe = np.zeros((L, D), np.float32)
    pe[:, 0::2] = np.sin(ang)
    pe[:, 1::2] = np.cos(ang)
    return pe


def _moving_mean(v, k=K_MA):
    pad = (k - 1) // 2
    vp = np.concatenate([np.repeat(v[:, :1], pad, 1), v,
                         np.repeat(v[:, -1:], pad, 1)], axis=1)
    c = np.cumsum(vp, axis=1, dtype=np.float32)
    c = np.concatenate([np.zeros_like(c[:, :1]), c], axis=1)
    return (c[:, k:] - c[:, :-k]) / np.float32(k)


def _gelu(x):
    return (x * 0.5 * (1.0 + erf(x / np.sqrt(2.0, dtype=np.float32)))).astype(
        np.float32)


def kernel(x, p, y, cls, tok_w, wq, bq, wo, bo, conv1_w, conv2_w,
           four_wr, four_wi, norm_g, norm_b, dec1_w, dec1_b, dec2_w, dec2_b):
    x = np.asarray(x, np.float32)
    # cls prepend + channel fold: (BE, L, CIN)
    xc = np.concatenate(
        [np.broadcast_to(np.asarray(cls, np.float32), (B, CH, 1, CIN)),
         np.transpose(x, (0, 2, 1, 3))], axis=2).reshape(BE, L, CIN)
    # circular conv k=3 as one matmul: [roll+1 | x | roll-1] @ [w0;w1;w2]
    x3 = np.concatenate([np.roll(xc, 1, axis=1), xc,
                         np.roll(xc, -1, axis=1)], axis=2).reshape(BE * L, 3 * CIN)
    tw = np.asarray(tok_w, np.float32)
    wtok = np.concatenate([tw[:, :, 0].T, tw[:, :, 1].T, tw[:, :, 2].T],
                          axis=0)                      # (192, 512)
    h = _mm_emb(x3, wtok).reshape(BE, L, D) + _pos_embed()[None]

    w_cplx = np.asarray(four_wr, np.float32) + 1j * np.asarray(four_wi, np.float32)
    for l in range(NL):
        q = _mm(h.reshape(BE * L, D), np.asarray(wq[l], np.float32).T)
        q = q + np.asarray(bq[l], np.float32)
        xq = q.reshape(BE, L, H, E).transpose(0, 2, 3, 1)       # (BE,H,E,L)
        x_ft = np.fft.rfft(xq, axis=-1)
        sel = np.einsum('bhim,hiom->bhom', x_ft[..., :M], w_cplx)
        out_ft = np.zeros(x_ft.shape, np.complex128)
        out_ft[..., :M] = sel
        a = np.fft.irfft(out_ft, n=L, axis=-1).astype(np.float32)
        a = a.reshape(BE, L, H * E)                              # torch .view
        a2 = _mm(a.reshape(BE * L, D), np.asarray(wo[l], np.float32).T)
        a2 = a2 + np.asarray(bo[l], np.float32)
        h = h + a2.reshape(BE, L, D)
        h = h - _moving_mean(h)
        yff = _mm_ff(h.reshape(BE * L, D),
                     np.asarray(conv1_w[l], np.float32).T,
                     np.asarray(conv2_w[l], np.float32).T)
        s2 = h + yff.reshape(BE, L, D)
        h = s2 - _moving_mean(s2)

    mu = np.mean(h, -1, keepdims=True)
    var = np.var(h, -1, keepdims=True)
    h = (h - mu) / np.sqrt(var + 1e-5) * np.asarray(norm_g, np.float32) \
        + np.asarray(norm_b, np.float32)
    z = np.mean(h, axis=1).reshape(B, CH * D)
    z = _gelu(z @ np.asarray(dec1_w, np.float32).T + np.asarray(dec1_b, np.float32))
    z = z @ np.asarray(dec2_w, np.float32).T + np.asarray(dec2_b, np.float32)
    return z[:, 0].astype(np.float32)
